# revision 22
# baseline (speedup 1.0000x reference)
"""Trainium2 Bass kernel for a GNN message-passing layer.

reference semantics (jax):
    src, dst = edge_index
    messages   = silu(concat(nodes[src], edge_features) @ mw1 + mb1)    # [E, D]
    aggregated = segment_sum(messages, dst, N)                          # [N, D]
    updated    = silu(concat(nodes, aggregated) @ uw1 + ub1) @ uw2 + ub2
    out        = nodes + updated

Distribution: destination-node partition across 8 cores. Nodes and MLP
weights are replicated; each core owns a contiguous 1/8 slice of the
(padded) node range, aggregates exactly the edges landing in its slice,
and runs the update MLP on its slice. No collectives.

Host-side work is limited to layout transforms of inputs (slicing,
padding, permutation of edge_features rows into the slot order, index
tables) — no float arithmetic.

Slot layout: each 128-node tile owns KMAX*128 edge slots (128 slots per
"edge tile"). Slots are filled with the tile's incoming edges — edges
whose source id falls in X-table window A (rows < 32768) first (KA edge
tiles), then window-B edges (KB tiles); leftover slots are pads with a
one-hot offset of -1 so their junk messages are scattered with weight 0.

Device pipeline per core:
  1. X = nodes @ mw1[:D] + mb1 into DRAM (transpose + matmul per tile).
  2. Per node tile: sequential DMA of the pre-permuted edge-feature
     rows; two dma_gather instructions (one per X-table window, int16
     indices) fetch all KMAX*128 X[src] rows. Per edge tile: PE
     transpose of ef tile, matmul with mw1[D:], add gathered X, SiLU,
     one-hot build (is_equal vs iota), scatter matmul into the node
     tile's PSUM accumulator.
  3. Update MLP in transposed space (4 node tiles per group), residual,
     transpose back, store.
"""

import math
import sys

sys.path.insert(0, "/opt/trn_rl_repo")

import numpy as np

import concourse.bacc as bacc
import concourse.mybir as mybir
import concourse.tile as tile
from concourse import bass_utils

P = 128
C = 8  # cores
WINA = 32768  # X-table window A rows (int16-addressable)

F32 = mybir.dt.float32
I16 = mybir.dt.int16
AF = mybir.ActivationFunctionType
OP = mybir.AluOpType


def _wrap16(stream):
    """[n] -> [16, n/16] wrapped layout: wrapped[i%16, i//16] = stream[i]."""
    return np.ascontiguousarray(stream.reshape(-1, 16).T)


def _host_prep(nodes, edge_index, edge_features, ntiles_pc):
    """Bucket edges by destination node tile, split by X-window, pad.

    Returns (ka, kb, per-core dict arrays).
    """
    N, D = nodes.shape
    E = edge_index.shape[1]
    NP_ = ntiles_pc * P
    N2 = NP_ * C
    ntiles = N2 // P

    src = edge_index[0].astype(np.int64)
    dst = edge_index[1].astype(np.int64)
    winb = (src >= WINA).astype(np.int64)
    # group by destination node tile, window-A edges first within each tile
    order = np.lexsort((winb, dst // P)).astype(np.int64)
    ds = dst[order]
    ss = src[order]
    wb = winb[order]

    tileid = ds // P
    counts = np.bincount(tileid, minlength=ntiles)
    countsB = np.bincount(tileid, weights=wb, minlength=ntiles).astype(np.int64)
    countsA = counts - countsB
    ka = max(1, int(math.ceil(countsA.max() / P)))
    kb = int(math.ceil(countsB.max() / P))
    kmax = ka + kb
    spt = kmax * P  # slots per tile
    SL = ntiles_pc * spt

    tile_start = np.zeros(ntiles + 1, np.int64)
    np.cumsum(counts, out=tile_start[1:])
    rank = np.arange(E, dtype=np.int64) - tile_start[tileid]
    # window-A edges come first within the tile; window-B start at ka*P
    slot_in_tile = np.where(wb == 0, rank, ka * P + rank - countsA[tileid])
    core = tileid // ntiles_pc
    t_local = tileid % ntiles_pc
    slot = t_local * spt + slot_in_tile

    dstoff = np.full((C, SL), -1.0, np.float32)
    dstoff[core, slot] = (ds - tileid * P).astype(np.float32)
    xidx = np.zeros((C, SL), np.int64)
    xidx[core, slot] = np.where(wb == 0, ss, ss - WINA)
    efsrc = np.full((C, SL), -1, np.int64)
    efsrc[core, slot] = order

    D_ = edge_features.shape[1]
    per_core = []
    for c in range(C):
        # permuted edge features (pads -> zero rows)
        efs = np.zeros((SL, D_), np.float32)
        valid = efsrc[c] >= 0
        efs[valid] = edge_features[efsrc[c][valid]]
        # int16 wrapped gather indices per node tile
        v = xidx[c].reshape(ntiles_pc, kmax * P)
        ia = np.zeros((P, ntiles_pc * ka * 8), np.int16)
        ib = np.zeros((P, max(1, ntiles_pc * kb * 8)), np.int16)
        for t in range(ntiles_pc):
            # wrapped [16, cols], replicated into all 8 Q7-core partition groups
            ia[:, t * ka * 8 : (t + 1) * ka * 8] = np.tile(
                _wrap16(v[t, : ka * P].astype(np.int16)), (8, 1)
            )
            if kb:
                ib[:, t * kb * 8 : (t + 1) * kb * 8] = np.tile(
                    _wrap16(v[t, ka * P :].astype(np.int16)), (8, 1)
                )
        dof = np.ascontiguousarray(
            dstoff[c].reshape(ntiles_pc * kmax, P).T
        )  # [P, ntiles*kmax]
        per_core.append(dict(efs=efs, idxA=ia, idxB=ib, dstoffT=dof))
    return ka, kb, per_core


def build_program(N2, D, ntiles_pc, ka, kb, debug=False):
    """Build the SPMD Bass program (identical across cores)."""
    assert D == P
    kmax = ka + kb
    nc = bacc.Bacc("TRN2", target_bir_lowering=False, debug=False, num_devices=C)
    NP_ = ntiles_pc * P
    SL = ntiles_pc * kmax * P

    d = lambda name, shape, dt=F32, kind="ExternalInput": nc.dram_tensor(
        name, shape, dt, kind=kind
    ).ap()

    nodes = d("nodes_pad", [N2, D])
    efs = d("efs", [SL, D])
    own = d("own_nodes", [NP_, D])
    idxA = d("idxA", [P, ntiles_pc * ka * 8], I16)
    idxB = d("idxB", [P, max(1, ntiles_pc * kb * 8)], I16)
    dstoff = d("dstoffT", [P, ntiles_pc * kmax])
    wt = d("wt", [D, D])
    wb_ = d("wb", [D, D])
    mb4 = d("mb4", [P, 4 * D])
    ua = d("ua", [D, D])
    ub = d("ub", [D, D])
    uw2 = d("uw2", [D, D])
    ub1c = d("ub1c", [P, 1])
    ub2c = d("ub2c", [P, 1])
    iota = d("iota", [P, P])
    ident = d("ident", [P, P])
    xdram = d("xdram", [N2, D], kind="ExternalOutput" if debug else "Internal")
    out = d("out_own", [NP_, D], kind="ExternalOutput")
    aggdbg = d("aggdbg", [P, ntiles_pc * D], kind="ExternalOutput") if debug else None
    if debug:
        xg0 = d("xg0", [P, ntiles_pc * kmax * D], kind="ExternalOutput")
        eg0 = d("eg0", [P, ntiles_pc * kmax * D], kind="ExternalOutput")

    with tile.TileContext(nc) as tc:
        with (
            tc.tile_pool(name="const", bufs=1) as cp,
            tc.tile_pool(name="sb", bufs=3) as sb,
            tc.tile_pool(name="big", bufs=2) as bigp,
        ):
            # ---- constants / index tables ----
            def load_const(ap, shape, dt=F32):
                t = cp.tile(shape, dt, tag=ap.name)
                nc.sync.dma_start(out=t[:], in_=ap[:])
                return t

            wt_s = load_const(wt, [D, D])
            wb_s = load_const(wb_, [D, D])
            mb4_s = load_const(mb4, [P, 4 * D])
            ua_s = load_const(ua, [D, D])
            ub_s = load_const(ub, [D, D])
            uw2_s = load_const(uw2, [D, D])
            ub1_s = load_const(ub1c, [P, 1])
            ub2_s = load_const(ub2c, [P, 1])
            iota_s = load_const(iota, [P, P])
            id_s = load_const(ident, [P, P])
            idxA_s = load_const(idxA, [P, ntiles_pc * ka * 8], I16)
            idxB_s = load_const(idxB, [P, max(1, ntiles_pc * kb * 8)], I16)
            doff_s = load_const(dstoff, [P, ntiles_pc * kmax])
            agg_all = cp.tile([P, ntiles_pc * D], F32, tag="agg_all")

            # ---- stage 1: X = nodes @ wt + mb1 ----
            pp1 = tc.tile_pool(name="psum1", bufs=2, space="PSUM")
            pp = pp1.__enter__()
            n2tiles = N2 // P
            for g in range(0, n2tiles, 4):
                gw = min(4, n2tiles - g)
                pT = pp.tile([P, 4 * P], F32, tag="pnT")
                for j in range(gw):
                    nt = sb.tile([P, D], F32, tag="nt")
                    nc.sync.dma_start(
                        out=nt[:], in_=nodes[(g + j) * P : (g + j + 1) * P, :]
                    )
                    nc.tensor.transpose(
                        out=pT[:, j * P : (j + 1) * P], in_=nt[:], identity=id_s[:]
                    )
                nT = sb.tile([P, 4 * P], F32, tag="nT4")
                nc.vector.tensor_copy(out=nT[:, : gw * P], in_=pT[:, : gw * P])
                pX = pp.tile([P, 4 * P], F32, tag="pX")
                for j in range(gw):
                    nc.tensor.matmul(
                        out=pX[:, j * P : (j + 1) * P],
                        lhsT=nT[:, j * P : (j + 1) * P],
                        rhs=wt_s[:],
                        start=True,
                        stop=True,
                    )
                x4 = sb.tile([P, 4 * P], F32, tag="x4")
                nc.vector.tensor_tensor(
                    out=x4[:, : gw * P],
                    in0=pX[:, : gw * P],
                    in1=mb4_s[:, : gw * P],
                    op=OP.add,
                )
                nc.sync.dma_start(
                    out=xdram[g * P : (g + gw) * P, :].rearrange(
                        "(j p) d -> p j d", p=P
                    ),
                    in_=x4[:, : gw * P].rearrange("p (j d) -> p j d", d=D),
                )
            pp1.__exit__(None, None, None)

            # ---- stage 2: edge pipeline ----
            pp2 = tc.tile_pool(name="psum2", bufs=2, space="PSUM")
            pp = pp2.__enter__()
            for t in range(ntiles_pc):
                eg = bigp.tile([P, kmax * D], F32, tag="eg")
                nc.sync.dma_start(
                    out=eg[:].rearrange("p (k d) -> p k d", d=D),
                    in_=efs[t * kmax * P : (t + 1) * kmax * P, :].rearrange(
                        "(k p) d -> p k d", p=P
                    ),
                )
                # dma_gather is limited to ~1024 indices per instruction
                GCH = 8
                xg = bigp.tile([P, kmax * D], F32, tag="xg")
                for k0 in range(0, ka, GCH):
                    kw = min(GCH, ka - k0)
                    nc.gpsimd.dma_gather(
                        out_ap=xg[:, k0 * D : (k0 + kw) * D].rearrange(
                            "p (k d) -> p k d", d=D
                        ),
                        in_ap=xdram[0:WINA, :] if N2 > WINA else xdram[:],
                        idxs_ap=idxA_s[:, (t * ka + k0) * 8 : (t * ka + k0 + kw) * 8],
                        num_idxs=kw * P,
                        num_idxs_reg=kw * P,
                        elem_size=D,
                    )
                for k0 in range(0, kb, GCH):
                    kw = min(GCH, kb - k0)
                    nc.gpsimd.dma_gather(
                        out_ap=xg[:, (ka + k0) * D : (ka + k0 + kw) * D].rearrange(
                            "p (k d) -> p k d", d=D
                        ),
                        in_ap=xdram[WINA:, :],
                        idxs_ap=idxB_s[:, (t * kb + k0) * 8 : (t * kb + k0 + kw) * 8],
                        num_idxs=kw * P,
                        num_idxs_reg=kw * P,
                        elem_size=D,
                    )
                if debug:
                    w0 = t * kmax * D
                    nc.sync.dma_start(out=xg0[:, w0 : w0 + kmax * D], in_=xg[:])
                    nc.sync.dma_start(out=eg0[:, w0 : w0 + kmax * D], in_=eg[:])
                pagg = pp.tile([P, D], F32, tag="pagg")
                for k in range(kmax):
                    pefT = pp.tile([P, P], F32, tag="pefT")
                    nc.tensor.transpose(
                        out=pefT[:], in_=eg[:, k * D : (k + 1) * D], identity=id_s[:]
                    )
                    efT = sb.tile([P, P], F32, tag="efT")
                    nc.vector.tensor_copy(out=efT[:], in_=pefT[:])
                    pmsg = pp.tile([P, D], F32, tag="pmsg")
                    nc.tensor.matmul(
                        out=pmsg[:], lhsT=efT[:], rhs=wb_s[:], start=True, stop=True
                    )
                    nc.vector.tensor_tensor(
                        out=pmsg[:],
                        in0=pmsg[:],
                        in1=xg[:, k * D : (k + 1) * D],
                        op=OP.add,
                    )
                    msg = sb.tile([P, D], F32, tag="msg")
                    nc.scalar.activation(out=msg[:], in_=pmsg[:], func=AF.Silu)
                    s_oh = sb.tile([P, P], F32, tag="s_oh")
                    nc.vector.tensor_tensor(
                        out=s_oh[:],
                        in0=doff_s[
                            :, t * kmax + k : t * kmax + k + 1
                        ].to_broadcast([P, P]),
                        in1=iota_s[:],
                        op=OP.is_equal,
                    )
                    nc.tensor.matmul(
                        out=pagg[:],
                        lhsT=s_oh[:],
                        rhs=msg[:],
                        start=(k == 0),
                        stop=(k == kmax - 1),
                    )
                nc.vector.tensor_copy(out=agg_all[:, t * D : (t + 1) * D], in_=pagg[:])
            if debug:
                nc.sync.dma_start(out=aggdbg[:], in_=agg_all[:])
            pp2.__exit__(None, None, None)

            # ---- stage 3: update MLP (transposed space, 4 node tiles/group) ----
            pp3 = tc.tile_pool(name="psum3", bufs=2, space="PSUM")
            pp = pp3.__enter__()
            for g in range(0, ntiles_pc, 4):
                gw = min(4, ntiles_pc - g)
                W = gw * P
                pT2 = pp.tile([P, 4 * P], F32, tag="ptr")
                for j in range(gw):
                    nt = sb.tile([P, D], F32, tag="nt")
                    nc.sync.dma_start(
                        out=nt[:], in_=own[(g + j) * P : (g + j + 1) * P, :]
                    )
                    nc.tensor.transpose(
                        out=pT2[:, j * P : (j + 1) * P], in_=nt[:], identity=id_s[:]
                    )
                ownT = sb.tile([P, 4 * P], F32, tag="ownT")
                nc.vector.tensor_copy(out=ownT[:, :W], in_=pT2[:, :W])
                pT3 = pp.tile([P, 4 * P], F32, tag="ptr")
                for j in range(gw):
                    nc.tensor.transpose(
                        out=pT3[:, j * P : (j + 1) * P],
                        in_=agg_all[:, (g + j) * D : (g + j + 1) * D],
                        identity=id_s[:],
                    )
                aggT = sb.tile([P, 4 * P], F32, tag="aggT")
                nc.vector.tensor_copy(out=aggT[:, :W], in_=pT3[:, :W])
                ph = pp.tile([P, 4 * P], F32, tag="ph")
                nc.tensor.matmul(
                    out=ph[:, :W], lhsT=ua_s[:], rhs=ownT[:, :W], start=True, stop=False
                )
                nc.tensor.matmul(
                    out=ph[:, :W], lhsT=ub_s[:], rhs=aggT[:, :W], start=False, stop=True
                )
                hT = sb.tile([P, 4 * P], F32, tag="hT")
                nc.scalar.activation(
                    out=hT[:, :W], in_=ph[:, :W], func=AF.Silu, bias=ub1_s[:, :1]
                )
                po = pp.tile([P, 4 * P], F32, tag="po")
                nc.tensor.matmul(
                    out=po[:, :W], lhsT=uw2_s[:], rhs=hT[:, :W], start=True, stop=True
                )
                oT = sb.tile([P, 4 * P], F32, tag="oT")
                nc.scalar.activation(
                    out=oT[:, :W], in_=po[:, :W], func=AF.Identity, bias=ub2_s[:, :1]
                )
                nc.vector.tensor_tensor(
                    out=oT[:, :W], in0=oT[:, :W], in1=ownT[:, :W], op=OP.add
                )
                pOut = pp.tile([P, 4 * P], F32, tag="ptr")
                for j in range(gw):
                    nc.tensor.transpose(
                        out=pOut[:, j * P : (j + 1) * P],
                        in_=oT[:, j * P : (j + 1) * P],
                        identity=id_s[:],
                    )
                ot = sb.tile([P, 4 * P], F32, tag="ot")
                nc.vector.tensor_copy(out=ot[:, :W], in_=pOut[:, :W])
                nc.sync.dma_start(
                    out=out[g * P : (g + gw) * P, :].rearrange("(j p) d -> p j d", p=P),
                    in_=ot[:, :W].rearrange("p (j d) -> p j d", d=D),
                )
            pp3.__exit__(None, None, None)

    nc.compile()
    return nc


def _run(nc, in_maps, trace=False):
    return bass_utils.run_bass_kernel_spmd(
        nc, in_maps, core_ids=list(range(C)), trace=trace
    )


def make_in_maps(nodes, edge_index, edge_features, mw1, mb1, uw1, ub1, uw2, ub2,
                 ntiles_pc):
    N, D = nodes.shape
    NP_ = ntiles_pc * P
    N2 = NP_ * C
    ka, kb, per_core = _host_prep(nodes, edge_index, edge_features, ntiles_pc)

    nodes_pad = np.zeros((N2, D), np.float32)
    nodes_pad[:N] = nodes
    iota = np.broadcast_to(np.arange(P, dtype=np.float32), (P, P)).copy()
    ident = np.eye(P, dtype=np.float32)
    mb4 = np.broadcast_to(np.tile(mb1.astype(np.float32), 4), (P, 4 * D)).copy()

    shared = dict(
        nodes_pad=nodes_pad,
        wt=np.ascontiguousarray(mw1[:D], np.float32),
        wb=np.ascontiguousarray(mw1[D:], np.float32),
        mb4=mb4,
        ua=np.ascontiguousarray(uw1[:D], np.float32),
        ub=np.ascontiguousarray(uw1[D:], np.float32),
        uw2=np.ascontiguousarray(uw2, np.float32),
        ub1c=np.ascontiguousarray(ub1.reshape(D, 1), np.float32),
        ub2c=np.ascontiguousarray(ub2.reshape(D, 1), np.float32),
        iota=iota,
        ident=ident,
    )
    in_maps = []
    for c in range(C):
        m = dict(shared)
        m["own_nodes"] = np.ascontiguousarray(nodes_pad[c * NP_ : (c + 1) * NP_])
        m["efs"] = per_core[c]["efs"]
        m["idxA"] = per_core[c]["idxA"]
        m["idxB"] = per_core[c]["idxB"]
        m["dstoffT"] = per_core[c]["dstoffT"]
        in_maps.append(m)
    return ka, kb, in_maps


def kernel(nodes, edge_index, edge_features, mw1, mb1, uw1, ub1, uw2, ub2):
    nodes = np.asarray(nodes, np.float32)
    edge_index = np.asarray(edge_index, np.int32)
    edge_features = np.asarray(edge_features, np.float32)
    N, D = nodes.shape
    ntiles_pc = math.ceil(N / (C * P))
    ka, kb, in_maps = make_in_maps(
        nodes, edge_index, edge_features, mw1, mb1, uw1, ub1, uw2, ub2, ntiles_pc
    )
    N2 = ntiles_pc * P * C
    nc = build_program(N2, D, ntiles_pc, ka, kb)
    res = _run(nc, in_maps)
    out = np.concatenate([res.results[c]["out_own"] for c in range(C)], axis=0)
    return out[:N].astype(np.float32)


if __name__ == "__main__":
    rng = np.random.default_rng(0)
    N, E, D = 4096, 16384, 128
    nodes = rng.standard_normal((N, D), dtype=np.float32)
    edge_index = rng.integers(0, N, (2, E)).astype(np.int32)
    ef = rng.standard_normal((E, D), dtype=np.float32)
    s2, s1 = 1 / np.sqrt(2 * D), 1 / np.sqrt(D)
    mw1 = rng.uniform(-s2, s2, (2 * D, D)).astype(np.float32)
    mb1 = rng.uniform(-s2, s2, D).astype(np.float32)
    uw1 = rng.uniform(-s2, s2, (2 * D, D)).astype(np.float32)
    ub1 = rng.uniform(-s2, s2, D).astype(np.float32)
    uw2 = rng.uniform(-s1, s1, (D, D)).astype(np.float32)
    ub2 = rng.uniform(-s1, s1, D).astype(np.float32)

    def silu(x):
        return x / (1 + np.exp(-x))

    def ref():
        src, dst = edge_index
        msg = silu(np.concatenate([nodes[src], ef], 1) @ mw1 + mb1)
        agg = np.zeros((N, D), np.float32)
        np.add.at(agg, dst, msg)
        upd = silu(np.concatenate([nodes, agg], 1) @ uw1 + ub1) @ uw2 + ub2
        return nodes + upd

    out = kernel(nodes, edge_index, ef, mw1, mb1, uw1, ub1, uw2, ub2)
    exp = ref()
    err = np.abs(out - exp).max() / np.abs(exp).max()
    print("tiny rel err:", err)


# revision 23
# speedup vs baseline: 1.4389x; 1.4389x over previous
"""Trainium2 Bass kernel for a GNN message-passing layer.

reference semantics (jax):
    src, dst = edge_index
    messages   = silu(concat(nodes[src], edge_features) @ mw1 + mb1)    # [E, D]
    aggregated = segment_sum(messages, dst, N)                          # [N, D]
    updated    = silu(concat(nodes, aggregated) @ uw1 + ub1) @ uw2 + ub2
    out        = nodes + updated

Distribution: destination-node partition across 8 cores. Nodes and MLP
weights are replicated; each core owns a contiguous 1/8 slice of the
(padded) node range, aggregates exactly the edges landing in its slice,
and runs the update MLP on its slice. No collectives.

Host-side work is limited to layout transforms of inputs (slicing,
padding, permutation of edge_features rows into slot order, per-tile
128x128 block transposes, index tables) — no float arithmetic.

Slot layout: each 128-node tile owns KMAX*128 edge slots (128 per "edge
tile"). Window-A edges (src < 32768) fill the first KA edge tiles, then
window-B edges (KB tiles); leftover slots are pads with one-hot offset
-1 so their junk messages scatter with weight 0.

Device pipeline per core:
  1. X = nodes @ mw1[:D] + mb1 into DRAM (matmul per tile; nodes arrive
     pre-transposed from the host).
  2. Per node tile: sequential DMA of pre-transposed edge-feature
     tiles; chunked dma_gather (int16, two X-table windows) fetches all
     KMAX*128 X[src] rows. Per 4-edge-tile chunk: 4 matmuls
     (lhsT=ef^T, rhs=mw1[D:]) into one PSUM group, one DVE add of the
     gathered X rows, one SiLU; per edge tile: one-hot build and a
     scatter matmul (lhsT=msg, rhs=one-hot) accumulating agg^T [d, j]
     in PSUM.
  3. Update MLP in transposed space (4 node tiles per group), residual,
     transpose back, store.

Optional bf16 paths (EF_BF16: message matmul operands; SC_BF16: scatter
matmul operands — the one-hot matrix is exact in bf16).
"""

import math
import sys

sys.path.insert(0, "/opt/trn_rl_repo")

import numpy as np

import concourse.bacc as bacc
import concourse.mybir as mybir
import concourse.tile as tile
from concourse import bass_utils

P = 128
C = 8  # cores
WINA = 32768  # X-table window A rows (int16-addressable)
GCH = 8  # dma_gather chunk (edge tiles per gather instruction)
EF_BF16 = True
SC_BF16 = True

F32 = mybir.dt.float32
BF16 = mybir.dt.bfloat16
I16 = mybir.dt.int16
AF = mybir.ActivationFunctionType
OP = mybir.AluOpType


def _wrap16(stream):
    """[n] -> [16, n/16] wrapped layout: wrapped[i%16, i//16] = stream[i]."""
    return np.ascontiguousarray(stream.reshape(-1, 16).T)


def _tileT(a):
    """[R*P, D] -> [R*D, P] with each 128-row block transposed."""
    R = a.shape[0] // P
    return np.ascontiguousarray(
        a.reshape(R, P, a.shape[1]).transpose(0, 2, 1)
    ).reshape(R * a.shape[1], P)


def _host_prep(nodes, edge_index, edge_features, ntiles_pc):
    """Bucket edges by destination node tile, split by X-window, pad."""
    N, D = nodes.shape
    E = edge_index.shape[1]
    NP_ = ntiles_pc * P
    N2 = NP_ * C
    ntiles = N2 // P

    src = edge_index[0].astype(np.int64)
    dst = edge_index[1].astype(np.int64)
    winb = (src >= WINA).astype(np.int64)
    # group by destination node tile, window-A edges first within each tile
    order = np.lexsort((winb, dst // P)).astype(np.int64)
    ds = dst[order]
    ss = src[order]
    wb = winb[order]

    tileid = ds // P
    counts = np.bincount(tileid, minlength=ntiles)
    countsB = np.bincount(tileid, weights=wb, minlength=ntiles).astype(np.int64)
    countsA = counts - countsB
    ka = max(1, int(math.ceil(countsA.max() / P)))
    kb = int(math.ceil(countsB.max() / P))
    kmax = ka + kb
    spt = kmax * P
    SL = ntiles_pc * spt

    tile_start = np.zeros(ntiles + 1, np.int64)
    np.cumsum(counts, out=tile_start[1:])
    rank = np.arange(E, dtype=np.int64) - tile_start[tileid]
    slot_in_tile = np.where(wb == 0, rank, ka * P + rank - countsA[tileid])
    core = tileid // ntiles_pc
    t_local = tileid % ntiles_pc
    slot = t_local * spt + slot_in_tile

    dstoff = np.full((C, SL), -1.0, np.float32)
    dstoff[core, slot] = (ds - tileid * P).astype(np.float32)
    xidx = np.zeros((C, SL), np.int64)
    xidx[core, slot] = np.where(wb == 0, ss, ss - WINA)
    efsrc = np.full((C, SL), -1, np.int64)
    efsrc[core, slot] = order

    per_core = []
    for c in range(C):
        efs = np.zeros((SL, D), np.float32)
        valid = efsrc[c] >= 0
        efs[valid] = edge_features[efsrc[c][valid]]
        efsT = _tileT(efs)  # [SL/P*D, P]: block tk rows = ef tile tk transposed
        v = xidx[c].reshape(ntiles_pc, kmax * P)
        ia = np.zeros((P, ntiles_pc * ka * 8), np.int16)
        ib = np.zeros((P, max(1, ntiles_pc * kb * 8)), np.int16)
        for t in range(ntiles_pc):
            ia[:, t * ka * 8 : (t + 1) * ka * 8] = np.tile(
                _wrap16(v[t, : ka * P].astype(np.int16)), (8, 1)
            )
            if kb:
                ib[:, t * kb * 8 : (t + 1) * kb * 8] = np.tile(
                    _wrap16(v[t, ka * P :].astype(np.int16)), (8, 1)
                )
        dof = np.ascontiguousarray(dstoff[c].reshape(ntiles_pc * kmax, P).T)
        per_core.append(dict(efsT=efsT, idxA=ia, idxB=ib, dstoffT=dof))
    return ka, kb, per_core


def build_program(N2, D, ntiles_pc, ka, kb, debug=False):
    """Build the SPMD Bass program (identical across cores)."""
    assert D == P
    kmax = ka + kb
    nc = bacc.Bacc("TRN2", target_bir_lowering=False, debug=False, num_devices=C)
    NP_ = ntiles_pc * P
    SL = ntiles_pc * kmax * P
    MDT = BF16 if SC_BF16 else F32  # messages / one-hot dtype for scatter

    d = lambda name, shape, dt=F32, kind="ExternalInput": nc.dram_tensor(
        name, shape, dt, kind=kind
    ).ap()

    nodesT = d("nodesT", [(N2 // P) * D, P])
    efsT = d("efsT", [(SL // P) * D, P])
    ownT_d = d("own_nodesT", [ntiles_pc * D, P])
    idxA = d("idxA", [P, ntiles_pc * ka * 8], I16)
    idxB = d("idxB", [P, max(1, ntiles_pc * kb * 8)], I16)
    dstoff = d("dstoffT", [P, ntiles_pc * kmax])
    wt = d("wt", [D, D])
    wb_ = d("wb", [D, D])
    mb4 = d("mb4", [P, 4 * D])
    ua = d("ua", [D, D])
    ub = d("ub", [D, D])
    uw2 = d("uw2", [D, D])
    ub1c = d("ub1c", [P, 1])
    ub2c = d("ub2c", [P, 1])
    iota = d("iota", [P, P])
    ident = d("ident", [P, P])
    xdram = d("xdram", [N2, D], kind="ExternalOutput" if debug else "Internal")
    out = d("out_own", [NP_, D], kind="ExternalOutput")
    aggdbg = d("aggdbg", [P, ntiles_pc * D], kind="ExternalOutput") if debug else None

    with tile.TileContext(nc) as tc:
        with (
            tc.tile_pool(name="const", bufs=1) as cp,
            tc.tile_pool(name="sb", bufs=3) as sb,
            tc.tile_pool(name="big", bufs=3) as bigp,
        ):
            def load_const(ap, shape, dt=F32):
                t = cp.tile(shape, dt, tag=ap.name)
                nc.sync.dma_start(out=t[:], in_=ap[:])
                return t

            wt_s = load_const(wt, [D, D])
            wb_s = load_const(wb_, [D, D])
            mb4_s = load_const(mb4, [P, 4 * D])
            ua_s = load_const(ua, [D, D])
            ub_s = load_const(ub, [D, D])
            uw2_s = load_const(uw2, [D, D])
            ub1_s = load_const(ub1c, [P, 1])
            ub2_s = load_const(ub2c, [P, 1])
            iota_s = load_const(iota, [P, P])
            id_s = load_const(ident, [P, P])
            idxA_s = load_const(idxA, [P, ntiles_pc * ka * 8], I16)
            idxB_s = load_const(idxB, [P, max(1, ntiles_pc * kb * 8)], I16)
            doff_s = load_const(dstoff, [P, ntiles_pc * kmax])
            aggT_all = cp.tile([P, ntiles_pc * D], F32, tag="aggT_all")
            if EF_BF16:
                wb16 = cp.tile([D, D], BF16, tag="wb16")
                nc.vector.tensor_copy(out=wb16[:], in_=wb_s[:])

            # ---- stage 1: X = nodes @ wt + mb1 ----
            pp1 = tc.tile_pool(name="psum1", bufs=2, space="PSUM")
            pp = pp1.__enter__()
            n2tiles = N2 // P
            for g in range(0, n2tiles, 4):
                gw = min(4, n2tiles - g)
                ntT = sb.tile([P, 4 * P], F32, tag="ntT")
                nc.sync.dma_start(
                    out=ntT[:, : gw * P].rearrange("p (j n) -> p j n", n=P),
                    in_=nodesT[g * D : (g + gw) * D, :].rearrange(
                        "(j d) n -> d j n", d=D
                    ),
                )
                pX = pp.tile([P, 4 * P], F32, tag="pX")
                for j in range(gw):
                    nc.tensor.matmul(
                        out=pX[:, j * P : (j + 1) * P],
                        lhsT=ntT[:, j * P : (j + 1) * P],
                        rhs=wt_s[:],
                        start=True,
                        stop=True,
                    )
                x4 = sb.tile([P, 4 * P], F32, tag="x4")
                nc.vector.tensor_tensor(
                    out=x4[:, : gw * P],
                    in0=pX[:, : gw * P],
                    in1=mb4_s[:, : gw * P],
                    op=OP.add,
                )
                nc.sync.dma_start(
                    out=xdram[g * P : (g + gw) * P, :].rearrange(
                        "(j p) d -> p j d", p=P
                    ),
                    in_=x4[:, : gw * P].rearrange("p (j d) -> p j d", d=D),
                )
            pp1.__exit__(None, None, None)

            # ---- stage 2: edge pipeline ----
            pp2 = tc.tile_pool(name="psum2", bufs=2, space="PSUM")
            pp = pp2.__enter__()
            for t in range(ntiles_pc):
                egT = bigp.tile([P, kmax * D], F32, tag="egT")
                nc.sync.dma_start(
                    out=egT[:].rearrange("p (k e) -> p k e", e=P),
                    in_=efsT[t * kmax * D : (t + 1) * kmax * D, :].rearrange(
                        "(k d) e -> d k e", d=D
                    ),
                )
                if EF_BF16:
                    egT16 = bigp.tile([P, kmax * D], BF16, tag="egT16")
                    nc.vector.tensor_copy(out=egT16[:], in_=egT[:])
                    eg_mm, wb_mm = egT16, wb16
                else:
                    eg_mm, wb_mm = egT, wb_s
                xg = bigp.tile([P, kmax * D], F32, tag="xg")
                for k0 in range(0, ka, GCH):
                    kw = min(GCH, ka - k0)
                    nc.gpsimd.dma_gather(
                        out_ap=xg[:, k0 * D : (k0 + kw) * D].rearrange(
                            "p (k d) -> p k d", d=D
                        ),
                        in_ap=xdram[0:WINA, :] if N2 > WINA else xdram[:],
                        idxs_ap=idxA_s[:, (t * ka + k0) * 8 : (t * ka + k0 + kw) * 8],
                        num_idxs=kw * P,
                        num_idxs_reg=kw * P,
                        elem_size=D,
                    )
                for k0 in range(0, kb, GCH):
                    kw = min(GCH, kb - k0)
                    nc.gpsimd.dma_gather(
                        out_ap=xg[:, (ka + k0) * D : (ka + k0 + kw) * D].rearrange(
                            "p (k d) -> p k d", d=D
                        ),
                        in_ap=xdram[WINA:, :],
                        idxs_ap=idxB_s[:, (t * kb + k0) * 8 : (t * kb + k0 + kw) * 8],
                        num_idxs=kw * P,
                        num_idxs_reg=kw * P,
                        elem_size=D,
                    )
                paggT = pp.tile([P, D], F32, tag="paggT")
                nch = math.ceil(kmax / 4)
                for ci in range(nch):
                    k0 = ci * 4
                    cw = min(4, kmax - k0)
                    W = cw * P
                    pmsg = pp.tile([P, 4 * P], F32, tag="pmsg")
                    for j in range(cw):
                        nc.tensor.matmul(
                            out=pmsg[:, j * P : (j + 1) * P],
                            lhsT=eg_mm[:, (k0 + j) * D : (k0 + j + 1) * D],
                            rhs=wb_mm[:],
                            start=True,
                            stop=True,
                        )
                    nc.vector.tensor_tensor(
                        out=pmsg[:, :W],
                        in0=pmsg[:, :W],
                        in1=xg[:, k0 * D : (k0 + cw) * D],
                        op=OP.add,
                    )
                    msg = sb.tile([P, 4 * P], MDT, tag="msg")
                    nc.scalar.activation(out=msg[:, :W], in_=pmsg[:, :W], func=AF.Silu)
                    for j in range(cw):
                        k = k0 + j
                        s_oh = sb.tile([P, P], MDT, tag="s_oh")
                        nc.vector.tensor_tensor(
                            out=s_oh[:],
                            in0=doff_s[
                                :, t * kmax + k : t * kmax + k + 1
                            ].to_broadcast([P, P]),
                            in1=iota_s[:],
                            op=OP.is_equal,
                        )
                        # aggT[d, j] += msg_k^T-contraction over e
                        nc.tensor.matmul(
                            out=paggT[:],
                            lhsT=msg[:, j * P : (j + 1) * P],
                            rhs=s_oh[:],
                            start=(k == 0),
                            stop=(k == kmax - 1),
                        )
                nc.vector.tensor_copy(out=aggT_all[:, t * D : (t + 1) * D], in_=paggT[:])
            if debug:
                nc.sync.dma_start(out=aggdbg[:], in_=aggT_all[:])
            pp2.__exit__(None, None, None)

            # ---- stage 3: update MLP (transposed space, 4 node tiles/group) ----
            pp3 = tc.tile_pool(name="psum3", bufs=2, space="PSUM")
            pp = pp3.__enter__()
            for g in range(0, ntiles_pc, 4):
                gw = min(4, ntiles_pc - g)
                W = gw * P
                ownT = sb.tile([P, 4 * P], F32, tag="ownT")
                nc.sync.dma_start(
                    out=ownT[:, :W].rearrange("p (j n) -> p j n", n=P),
                    in_=ownT_d[g * D : (g + gw) * D, :].rearrange(
                        "(j d) n -> d j n", d=D
                    ),
                )
                ph = pp.tile([P, 4 * P], F32, tag="ph")
                nc.tensor.matmul(
                    out=ph[:, :W], lhsT=ua_s[:], rhs=ownT[:, :W], start=True, stop=False
                )
                nc.tensor.matmul(
                    out=ph[:, :W],
                    lhsT=ub_s[:],
                    rhs=aggT_all[:, g * D : g * D + W],
                    start=False,
                    stop=True,
                )
                hT = sb.tile([P, 4 * P], F32, tag="hT")
                nc.scalar.activation(
                    out=hT[:, :W], in_=ph[:, :W], func=AF.Silu, bias=ub1_s[:, :1]
                )
                po = pp.tile([P, 4 * P], F32, tag="po")
                nc.tensor.matmul(
                    out=po[:, :W], lhsT=uw2_s[:], rhs=hT[:, :W], start=True, stop=True
                )
                oT = sb.tile([P, 4 * P], F32, tag="oT")
                nc.scalar.activation(
                    out=oT[:, :W], in_=po[:, :W], func=AF.Identity, bias=ub2_s[:, :1]
                )
                nc.vector.tensor_tensor(
                    out=oT[:, :W], in0=oT[:, :W], in1=ownT[:, :W], op=OP.add
                )
                pOut = pp.tile([P, 4 * P], F32, tag="ptr")
                for j in range(gw):
                    nc.tensor.transpose(
                        out=pOut[:, j * P : (j + 1) * P],
                        in_=oT[:, j * P : (j + 1) * P],
                        identity=id_s[:],
                    )
                ot = sb.tile([P, 4 * P], F32, tag="ot")
                nc.vector.tensor_copy(out=ot[:, :W], in_=pOut[:, :W])
                nc.sync.dma_start(
                    out=out[g * P : (g + gw) * P, :].rearrange("(j p) d -> p j d", p=P),
                    in_=ot[:, :W].rearrange("p (j d) -> p j d", d=D),
                )
            pp3.__exit__(None, None, None)

    nc.compile()
    return nc


def _run(nc, in_maps, trace=False):
    return bass_utils.run_bass_kernel_spmd(
        nc, in_maps, core_ids=list(range(C)), trace=trace
    )


def make_in_maps(nodes, edge_index, edge_features, mw1, mb1, uw1, ub1, uw2, ub2,
                 ntiles_pc):
    N, D = nodes.shape
    NP_ = ntiles_pc * P
    N2 = NP_ * C
    ka, kb, per_core = _host_prep(nodes, edge_index, edge_features, ntiles_pc)

    nodes_pad = np.zeros((N2, D), np.float32)
    nodes_pad[:N] = nodes
    nodesT = _tileT(nodes_pad)
    iota = np.broadcast_to(np.arange(P, dtype=np.float32), (P, P)).copy()
    ident = np.eye(P, dtype=np.float32)
    mb4 = np.broadcast_to(np.tile(mb1.astype(np.float32), 4), (P, 4 * D)).copy()

    shared = dict(
        nodesT=nodesT,
        wt=np.ascontiguousarray(mw1[:D], np.float32),
        wb=np.ascontiguousarray(mw1[D:], np.float32),
        mb4=mb4,
        ua=np.ascontiguousarray(uw1[:D], np.float32),
        ub=np.ascontiguousarray(uw1[D:], np.float32),
        uw2=np.ascontiguousarray(uw2, np.float32),
        ub1c=np.ascontiguousarray(ub1.reshape(D, 1), np.float32),
        ub2c=np.ascontiguousarray(ub2.reshape(D, 1), np.float32),
        iota=iota,
        ident=ident,
    )
    in_maps = []
    for c in range(C):
        m = dict(shared)
        m["own_nodesT"] = _tileT(
            np.ascontiguousarray(nodes_pad[c * NP_ : (c + 1) * NP_])
        )
        m["efsT"] = per_core[c]["efsT"]
        m["idxA"] = per_core[c]["idxA"]
        m["idxB"] = per_core[c]["idxB"]
        m["dstoffT"] = per_core[c]["dstoffT"]
        in_maps.append(m)
    return ka, kb, in_maps


def kernel(nodes, edge_index, edge_features, mw1, mb1, uw1, ub1, uw2, ub2):
    nodes = np.asarray(nodes, np.float32)
    edge_index = np.asarray(edge_index, np.int32)
    edge_features = np.asarray(edge_features, np.float32)
    N, D = nodes.shape
    ntiles_pc = math.ceil(N / (C * P))
    ka, kb, in_maps = make_in_maps(
        nodes, edge_index, edge_features, mw1, mb1, uw1, ub1, uw2, ub2, ntiles_pc
    )
    N2 = ntiles_pc * P * C
    nc = build_program(N2, D, ntiles_pc, ka, kb)
    res = _run(nc, in_maps)
    out = np.concatenate([res.results[c]["out_own"] for c in range(C)], axis=0)
    return out[:N].astype(np.float32)


if __name__ == "__main__":
    rng = np.random.default_rng(0)
    N, E, D = 4096, 16384, 128
    nodes = rng.standard_normal((N, D), dtype=np.float32)
    edge_index = rng.integers(0, N, (2, E)).astype(np.int32)
    ef = rng.standard_normal((E, D), dtype=np.float32)
    s2, s1 = 1 / np.sqrt(2 * D), 1 / np.sqrt(D)
    mw1 = rng.uniform(-s2, s2, (2 * D, D)).astype(np.float32)
    mb1 = rng.uniform(-s2, s2, D).astype(np.float32)
    uw1 = rng.uniform(-s2, s2, (2 * D, D)).astype(np.float32)
    ub1 = rng.uniform(-s2, s2, D).astype(np.float32)
    uw2 = rng.uniform(-s1, s1, (D, D)).astype(np.float32)
    ub2 = rng.uniform(-s1, s1, D).astype(np.float32)

    def silu(x):
        return x / (1 + np.exp(-x))

    def ref():
        src, dst = edge_index
        msg = silu(np.concatenate([nodes[src], ef], 1) @ mw1 + mb1)
        agg = np.zeros((N, D), np.float32)
        np.add.at(agg, dst, msg)
        upd = silu(np.concatenate([nodes, agg], 1) @ uw1 + ub1) @ uw2 + ub2
        return nodes + upd

    out = kernel(nodes, edge_index, ef, mw1, mb1, uw1, ub1, uw2, ub2)
    exp = ref()
    err = np.abs(out - exp).max() / np.abs(exp).max()
    print("tiny rel err:", err)
